# revision 3
# baseline (speedup 1.0000x reference)
"""DepthSelfAttention Trainium2 kernel — position-major rewrite.

8-core data-parallel SPMD. Contract: kernel(**inputs) takes FULL unsharded
numpy inputs, returns the FULL [4, 2048, 2048] fp32 output.

Layout strategy (vs the combo-major baseline):
  * positions on the partition dim everywhere (tiles of 128 positions)
  * TensorE does ONLY the 4 projections (+16 small transposes/tile)
  * rmsnorm / RoPE / scores / softmax / attn*V run on DVE+Scalar with
    broadcast access patterns -> no auxiliary matmuls
  * DVE work is batched across all 8 depths per tile, f16 contiguous
    outputs keep the DVE in its 2x/4x fast modes
  * two passes per core: A = Q+K+scores+attn (wq,wk resident),
    B = V+y+yT+O per tile with lag-1 skew (wv+wp resident)
"""

import sys

sys.path.insert(0, "/opt/trn_rl_repo")

from contextlib import ExitStack, nullcontext

import numpy as np

import concourse.bass as bass
import concourse.tile as tile
from concourse import bacc, mybir

F16 = mybir.dt.float16
F32 = mybir.dt.float32

DIM = 2048
NH = 16
NKV = 4
HD = 128
DEP = 8  # 7 history + current
NCORES = 8
EPS = 1.1920929e-07
SCALE = 1.0 / float(np.sqrt(HD))
ROPE_BASE = 10000.0
MAX_DEPTH = 16
DC = DIM // 128  # 16 contraction chunks
# depth processing order: slot j holds depth ORD[j]; depth 7 (= x itself)
# first because its kvt tile doubles as the Q-projection input
ORD = [7, 0, 1, 2, 3, 4, 5, 6]

LAST_RESULTS = None
_PROGRAM_CACHE = {}


# ---------------------------------------------------------------- host tables
def _rope_tables():
    inv_freq = 1.0 / ROPE_BASE ** (
        np.arange(0, HD, 2, dtype=np.float32) / HD
    )  # [64]
    pos = np.arange(MAX_DEPTH, dtype=np.float32)
    rpos = np.arange(MAX_DEPTH - 1, -1, -1, dtype=np.float32)
    fw = np.outer(pos, inv_freq)
    rv = np.outer(rpos, inv_freq)
    return np.cos(fw), np.sin(fw), np.cos(rv), np.sin(rv)  # each [16, 64]


def _host_constants(q_gain):
    DCOS, DSIN, RCOS, RSIN = _rope_tables()
    d = DEP - 1  # query position index

    # pair-table layout: rot = [x1*c + x2*s | x1*nrs + x2*rc]  (nrs = -rev_sin)
    #   tabA = [c | nrs] multiplies x1 for both halves,
    #   tabB = [s | rc ] multiplies x2 for both halves.
    qtab = np.stack(
        [np.concatenate([DCOS[d], -RSIN[d]]), np.concatenate([DSIN[d], RCOS[d]])]
    )  # [2(A/B), 128]
    qtab = np.broadcast_to(qtab, (128, 2, HD))

    ktab = np.stack(
        [
            np.concatenate([DCOS[:DEP], -RSIN[:DEP]], axis=1),
            np.concatenate([DSIN[:DEP], RCOS[:DEP]], axis=1),
        ]
    )  # [2(A/B), 8, 128]
    ktab = ktab[:, ORD, :]  # slot order (slot j holds depth ORD[j])
    ktab = np.broadcast_to(ktab, (128, 2, DEP, HD))

    ident = np.eye(128, dtype=np.float32)

    cg = SCALE * np.asarray(q_gain, np.float64)  # [16]
    cg = np.broadcast_to(cg, (128, NH))

    f16 = np.float16
    return {
        "qtab": np.ascontiguousarray(qtab).astype(f16),
        "ktab": np.ascontiguousarray(ktab).astype(f16),
        "ident": ident.astype(f16),
        "cg": np.ascontiguousarray(cg).astype(np.float32),
        "epsb": np.full((128, 1), EPS, np.float32),
    }


def _prep_weights(Wq, Wk, Wv, Wproj):
    f16 = np.float16

    def t_part(w):  # [out, din] -> [128, din//128, out]
        wt = np.ascontiguousarray(np.asarray(w, np.float32).T)  # [din, out]
        no = wt.shape[1]
        return np.ascontiguousarray(
            wt.reshape(wt.shape[0] // 128, 128, no).transpose(1, 0, 2)
        ).astype(f16)

    return {
        "wq": t_part(Wq),  # [128, 16, 2048]
        "wk": t_part(Wk),  # [128, 16, 512]
        "wv": t_part(Wv),  # [128, 16, 512]
        "wp": t_part(Wproj),  # [128, 16, 2048]
    }


def _prep_kvt(x, depth_history):
    """-> kvt [DEP, NTG, 128p, DC, 128n] fp16 where NTG = N/128 global tiles.

    kvt[dep, T, p, c, n] = kv[T*128+n, dep, c*128+p]
    """
    B, S, D = x.shape
    N = B * S
    xf = np.asarray(x, np.float32).reshape(N, 1, D)
    dh = np.asarray(depth_history, np.float32).reshape(N, DEP - 1, D)
    kv = np.concatenate([dh, xf], axis=1).astype(np.float16)  # [N, 8, D]
    NTG = N // 128
    kvt = (
        kv.transpose(1, 0, 2)  # [DEP, N, D]
        .reshape(DEP, NTG, 128, DC, 128)  # [dep, T, n, c, p]
        .transpose(0, 1, 4, 3, 2)  # [dep, T, p, c, n]
    )
    return np.ascontiguousarray(kvt)


# ---------------------------------------------------------------- device code
def build_program(npc, reps=1):
    """One-core SPMD program for npc positions (NT = npc/128 tiles)."""
    NT = npc // 128
    assert npc % 128 == 0

    nc = bacc.Bacc()
    kvt_d = nc.declare_dram_parameter(
        "kvt", [DEP, NT, 128, DC, 128], F16, isOutput=False
    )
    wq_d = nc.declare_dram_parameter("wq", [128, DC, DIM], F16, isOutput=False)
    wk_d = nc.declare_dram_parameter("wk", [128, DC, 512], F16, isOutput=False)
    wv_d = nc.declare_dram_parameter("wv", [128, DC, 512], F16, isOutput=False)
    wp_d = nc.declare_dram_parameter("wp", [128, DC, DIM], F16, isOutput=False)
    qtab_d = nc.declare_dram_parameter("qtab", [128, 2, HD], F16, isOutput=False)
    ktab_d = nc.declare_dram_parameter(
        "ktab", [128, 2, DEP, HD], F16, isOutput=False
    )
    id_d = nc.declare_dram_parameter("ident", [128, 128], F16, isOutput=False)
    cg_d = nc.declare_dram_parameter("cg", [128, NH], F32, isOutput=False)
    epsb_d = nc.declare_dram_parameter("epsb", [128, 1], F32, isOutput=False)
    out_d = nc.declare_dram_parameter("out", [npc, DIM], F32, isOutput=True)

    AF = mybir.ActivationFunctionType
    AX = mybir.AxisListType
    OP = mybir.AluOpType

    with tile.TileContext(nc) as tc, ExitStack() as top:
        const = top.enter_context(tc.tile_pool(name="const", bufs=1))

        def load_const(dram, shape, name):
            t = const.tile(shape, dram.dtype, name=name)
            nc.sync.dma_start(t[:], dram[:])
            return t

        qtab_sb = load_const(qtab_d, [128, 2, HD], "qtab_sb")
        ktab_sb = load_const(ktab_d, [128, 2, DEP, HD], "ktab_sb")
        id_sb = load_const(id_d, [128, 128], "id_sb")
        cg_sb = load_const(cg_d, [128, NH], "cg_sb")
        epsb_sb = load_const(epsb_d, [128, 1], "epsb_sb")

        rep_ctx = tc.For_i(0, reps, 1) if reps > 4 else nullcontext(None)
        with rep_ctx:
         for _rep in range(reps if reps <= 4 else 1):
          with (
              tc.tile_pool(name="keep", bufs=1) as keep,
              tc.tile_pool(name="wvp", bufs=1) as wvp,
              tc.tile_pool(name="kvbp", bufs=3) as kvbp,
          ):
            attn_keep = {}
            # wv lives in a rep-level pool (disjoint from the phase-A pools)
            # so its DMA (emitted at tile1) overlaps phase A instead of
            # gating phase B's start
            wv_sb = wvp.tile([128, DC, 512], F16, name="wv_sb")

            # ================= Phase A: Q + K + scores + attn =================
            with (
                tc.tile_pool(name="wqp", bufs=1) as wqp,
                tc.tile_pool(name="wkp", bufs=1) as wkp,
                tc.tile_pool(name="kv7p", bufs=2) as kv7p,
                tc.tile_pool(name="kvp", bufs=3) as kvp,
                tc.tile_pool(name="qps", bufs=2, space="PSUM") as qps,
                tc.tile_pool(name="kps", bufs=2, space="PSUM") as kps,
                tc.tile_pool(name="qsp", bufs=2) as qsp,
                tc.tile_pool(name="wrk", bufs=1) as wrk,
            ):
                # DMA issue order tuned for the serial queue: tile0's kv7
                # first, then wq in chunks interleaved with wk, so tile0's
                # Q-proj starts after ~2 chunks instead of after all weights.
                wq_sb = wqp.tile([128, DC, DIM], F16, name="wq_sb")
                wk_sb = wkp.tile([128, DC, 512], F16, name="wk_sb")
                kv7_first = kv7p.tile([128, DC, 128], F16, name="kv7", tag="kv7")
                nc.sync.dma_start(kv7_first[:], kvt_d[DEP - 1, 0])
                for oc in range(4):
                    nc.sync.dma_start(
                        wq_sb[:, :, oc * 512 : (oc + 1) * 512],
                        wq_d[:, :, oc * 512 : (oc + 1) * 512],
                    )
                    if oc == 1:
                        nc.sync.dma_start(wk_sb[:], wk_d[:])

                for t in range(NT):
                    if t == 0:
                        kv7 = kv7_first
                    else:
                        kv7 = kv7p.tile([128, DC, 128], F16, name="kv7", tag="kv7")
                        nc.sync.dma_start(kv7[:], kvt_d[DEP - 1, t])

                    q_sb = qsp.tile([128, DIM], F16, name="q_sb", tag="q")
                    qsq = wrk.tile([128, DEP // 2, NH, HD], F16, name="qsq",
                                   tag="big")
                    t_a = wrk.tile([128, DEP * 512], F16, name="t_a", tag="t_a")
                    t_b = wrk.tile([128, DEP * 512], F16, name="t_b", tag="t_b")
                    qrot = qsp.tile([128, NH, HD], F16, name="qrot", tag="qrot",
                                    bufs=1)
                    k_all = qsp.tile([128, DEP, 512], F16, name="k_all", tag="kall")
                    ksq = wrk.tile([128, DEP, 512], F16, name="ksq", tag="t_c")
                    ssq_k = wrk.tile([128, DEP, NKV], F16, name="ssq_k", tag="sk")
                    scores = wrk.tile([128, DEP, NH], F16, name="scores", tag="sc")
                    dq = wrk.tile([128, NH], F32, name="dq", tag="dq")
                    HB = DEP // 2  # slots per half-batch

                    def emit_q():
                        # ---- Q projection: q_sb [128 pos, 2048] f16 ----
                        for oc in range(4):
                            qp = qps.tile([128, 512], F32, name="q_ps", tag="qps")
                            for c in range(DC):
                                nc.tensor.matmul(
                                    qp[:],
                                    kv7[:, c, :],
                                    wq_sb[:, c, oc * 512 : (oc + 1) * 512],
                                    start=(c == 0),
                                    stop=(c == DC - 1),
                                )
                            nc.scalar.copy(
                                q_sb[:, oc * 512 : (oc + 1) * 512], qp[:]
                            )

                        # ---- q rmsnorm stats + rope (DVE, f16 fast modes) ----
                        qsq_v = qsq[:].rearrange("p d h x -> p (d h x)")[:, :DIM]
                        nc.vector.tensor_mul(qsq_v, q_sb[:], q_sb[:])
                        ssq_q = wrk.tile([128, NH], F16, name="ssq_q", tag="ssq_q")
                        with nc.allow_low_precision("f16 sumsq, values O(100)"):
                            nc.vector.tensor_reduce(
                                ssq_q[:],
                                qsq_v.rearrange("p (h x) -> p h x", h=NH),
                                axis=AX.X,
                                op=OP.add,
                            )
                        nc.scalar.activation(
                            dq[:], ssq_q[:], AF.Sqrt, bias=epsb_sb[:],
                            scale=1.0 / HD,
                        )

                        q_v = q_sb[:].rearrange(
                            "p (h two j) -> p h two j", two=2, j=64
                        )
                        t_qa = t_a[:, :DIM].rearrange("p (h x) -> p h x", h=NH)
                        t_qb = t_b[:, :DIM].rearrange("p (h x) -> p h x", h=NH)
                        nc.vector.tensor_mul(
                            t_qa,
                            q_v[:, :, 0:1, :].broadcast_to((128, NH, 2, 64)),
                            qtab_sb[:, 0:1, :].broadcast_to((128, NH, HD)),
                        )
                        nc.vector.tensor_mul(
                            t_qb,
                            q_v[:, :, 1:2, :].broadcast_to((128, NH, 2, 64)),
                            qtab_sb[:, 1:2, :].broadcast_to((128, NH, HD)),
                        )
                        nc.vector.tensor_add(qrot[:], t_qa, t_qb)

                    def emit_kdve_half(ha):
                        sl = slice(HB * ha, HB * (ha + 1))
                        nc.vector.tensor_mul(
                            ksq[:, sl, :], k_all[:, sl, :], k_all[:, sl, :]
                        )
                        with nc.allow_low_precision("f16 sumsq, values O(100)"):
                            nc.vector.tensor_reduce(
                                ssq_k[:, sl, :],
                                ksq[:, sl, :].rearrange(
                                    "p d (g x) -> p d g x", g=NKV
                                ),
                                axis=AX.X,
                                op=OP.add,
                            )
                        # walrus DVE ISA allows at most 3 free AP dims, so
                        # (d,g) is pre-merged into one dim everywhere below
                        k_v = k_all[:, sl, :].rearrange(
                            "p d (g two j) -> p (d g) two j", two=2, j=64
                        )
                        t_ka = t_a[:, : HB * 512]
                        t_kb = t_b[:, : HB * 512]
                        nc.vector.tensor_mul(
                            t_ka,
                            k_v[:, :, 0:1, :].broadcast_to(
                                (128, HB * NKV, 2, 64)
                            ),
                            ktab_sb[:, 0, sl, :].unsqueeze(2).broadcast_to(
                                (128, HB, NKV, HD)
                            ),
                        )
                        nc.vector.tensor_mul(
                            t_kb,
                            k_v[:, :, 1:2, :].broadcast_to(
                                (128, HB * NKV, 2, 64)
                            ),
                            ktab_sb[:, 1, sl, :].unsqueeze(2).broadcast_to(
                                (128, HB, NKV, HD)
                            ),
                        )
                        # krot overwrites ksq's slots (already consumed)
                        krot_flat = ksq[:, sl, :].rearrange("p d x -> p (d x)")
                        nc.vector.tensor_add(krot_flat, t_ka, t_kb)

                        prod = qsq[:, :HB, :, :]  # big tmp tile region
                        nc.vector.tensor_mul(
                            prod.rearrange("p d h x -> p (d h x)"),
                            qrot[:]
                            .rearrange("p h x -> p (h x)")
                            .unsqueeze(1)
                            .broadcast_to((128, HB, NH * HD)),
                            ksq[:, sl, :]
                            .rearrange("p d (g x) -> p (d g) x", g=NKV)
                            .unsqueeze(2)
                            .broadcast_to((128, HB * NKV, 4, HD)),
                        )
                        # tree-fold the 128-wide dot products (2x mode; a
                        # single tensor_reduce runs at 1x and is ~2x slower)
                        fsrc = prod.rearrange("p d h x -> p (d h) x")
                        width = HD
                        ping, pong = t_a, t_b
                        while width > 2:
                            half = width // 2
                            fdst = ping[:, : HB * NH * half].rearrange(
                                "p (c x) -> p c x", c=HB * NH
                            )
                            nc.vector.tensor_add(
                                fdst, fsrc[:, :, :half], fsrc[:, :, half:width]
                            )
                            fsrc = fdst
                            ping, pong = pong, ping
                            width = half
                        nc.vector.tensor_add(
                            scores[:, sl, :]
                            .rearrange("p d h -> p (d h)")
                            .unsqueeze(2),
                            fsrc[:, :, 0:1],
                            fsrc[:, :, 1:2],
                        )

                    def emit_kmm(mid_dve):
                        # ---- K projections (8 depths, slot order ORD) ----
                        for j, dep in enumerate(ORD):
                            if j == 0:
                                kvt = kv7
                            else:
                                kvt = kvp.tile(
                                    [128, DC, 128], F16, name="kvt", tag="kvt"
                                )
                                nc.sync.dma_start(kvt[:], kvt_d[dep, t])
                            kp = kps.tile([128, 512], F32, name="k_ps", tag="kps")
                            for c in range(DC):
                                nc.tensor.matmul(
                                    kp[:],
                                    kvt[:, c, :],
                                    wk_sb[:, c, :],
                                    start=(c == 0),
                                    stop=(c == DC - 1),
                                )
                            nc.scalar.copy(k_all[:, j, :], kp[:])
                            if mid_dve and j == HB - 1:
                                emit_kdve_half(0)

                    if t == 0:
                        # tile0: K first so PE starts on wk+kv7 while the wq
                        # chunks are still streaming in; DVE K-halves run
                        # after Q (they need qrot)
                        emit_kmm(mid_dve=False)
                        emit_q()
                        emit_kdve_half(0)
                        emit_kdve_half(1)
                    else:
                        emit_q()
                        emit_kmm(mid_dve=True)
                        emit_kdve_half(1)
                    if t == min(1, NT - 1):
                        # wv prefetch; queue-positioned here so it does not
                        # delay the startup-critical wq/kv7 transfers
                        nc.sync.dma_start(wv_sb[:], wv_d[:])

                    # ---- softmax over depth (deferred rms normalization) ----
                    dk = wrk.tile([128, DEP, NKV], F32, name="dk", tag="dk")
                    nc.scalar.activation(
                        dk[:], ssq_k[:], AF.Sqrt, bias=epsb_sb[:], scale=1.0 / HD
                    )
                    dprod = wrk.tile([128, NH, DEP], F32, name="dprod", tag="dp")
                    nc.vector.tensor_mul(
                        dprod[:].rearrange("p (g r) x -> p g r x", g=NKV),
                        dq[:]
                        .rearrange("p (g r) -> p g r", g=NKV)
                        .unsqueeze(3)
                        .broadcast_to((128, NKV, 4, DEP)),
                        dk[:]
                        .rearrange("p d g -> p g d")
                        .unsqueeze(2)
                        .broadcast_to((128, NKV, 4, DEP)),
                    )
                    rden = wrk.tile([128, NH, DEP], F32, name="rden", tag="rd")
                    nc.vector.reciprocal(rden[:], dprod[:])
                    factor = wrk.tile([128, NH, DEP], F32, name="factor", tag="fa")
                    nc.vector.tensor_mul(
                        factor[:],
                        rden[:],
                        cg_sb[:].unsqueeze(2).broadcast_to((128, NH, DEP)),
                    )
                    scaled = wrk.tile([128, NH, DEP], F32, name="scaled", tag="sd")
                    nc.vector.tensor_mul(
                        scaled[:],
                        scores[:].rearrange("p d h -> p h d"),
                        factor[:],
                    )
                    epx = wrk.tile([128, NH, DEP], F32, name="epx", tag="ep")
                    nc.scalar.activation(epx[:], scaled[:], AF.Exp)
                    dsum = wrk.tile([128, NH], F32, name="dsum", tag="ds")
                    nc.vector.tensor_reduce(
                        dsum[:], epx[:], axis=AX.X, op=OP.add
                    )
                    rsum = wrk.tile([128, NH], F32, name="rsum", tag="rs")
                    nc.vector.reciprocal(rsum[:], dsum[:])
                    attn = keep.tile(
                        [128, NH, DEP], F16, name=f"attn{t}", tag=f"attn{t}"
                    )
                    attn_keep[t] = attn
                    nc.vector.tensor_mul(
                        attn[:],
                        epx[:],
                        rsum[:].unsqueeze(2).broadcast_to((128, NH, DEP)),
                    )

            # ============ Phase B: V + y + yT + O (lag-1 skew) ============
            # pool-open order matters: ybp first so it lands in wq's freed
            # region (whose users finish early) rather than overlapping the
            # wrk region still being read by phase A's DVE tail; vps first
            # among PSUM pools so it lands on banks phase A never used.
            with (
                tc.tile_pool(name="vps", bufs=2, space="PSUM") as vps,
                tc.tile_pool(name="tps", bufs=2, space="PSUM") as tps,
                tc.tile_pool(name="ybp", bufs=2) as ybp,
                tc.tile_pool(name="wpp", bufs=1) as wpp,
                tc.tile_pool(name="ops", bufs=2, space="PSUM") as opsp,
                tc.tile_pool(name="osb", bufs=3) as osbp,
            ):
                # wp prefetch is emitted in chunks after tile0's kvt loads so
                # it doesn't block the V-path stream on the serial DMA queue;
                # first use (O-proj of tile0) is one tile later.
                wp_sb = wpp.tile([128, DC, DIM], F16, name="wp_sb")

                y_of = {}
                yt_of = {}

                def emit_transpose(t):
                    y = y_of.pop(t)
                    tp = tps.tile([128, NH, 128], F16, name="t_ps", tag="tps")
                    for h in range(NH):
                        nc.tensor.transpose(
                            tp[:, h, :], y[:, h * 128 : (h + 1) * 128], id_sb[:]
                        )
                    yt = ybp.tile([128, DC, 128], F16, name="yt", tag="yt")
                    yt_of[t] = yt
                    nc.scalar.copy(yt[:], tp[:])

                def emit_oproj(t):
                    yt = yt_of.pop(t)
                    row = t * 128
                    for og in range(4):
                        op_t = opsp.tile([128, 512], F32, name="o_ps", tag="ops")
                        for c in range(DC):
                            nc.tensor.matmul(
                                op_t[:],
                                yt[:, c, :],
                                wp_sb[:, c, og * 512 : (og + 1) * 512],
                                start=(c == 0),
                                stop=(c == DC - 1),
                            )
                        ost = osbp.tile([128, 512], F32, name="ost", tag="ost")
                        nc.scalar.copy(ost[:], op_t[:])
                        nc.sync.dma_start(
                            out_d[row : row + 128, og * 512 : (og + 1) * 512],
                            ost[:],
                        )

                HB = DEP // 2
                for t in range(NT):
                    attn = attn_keep[t]
                    # v_t: depth-innermost V staging so the big attn*v multiply
                    # keeps a stride-1 last dim (DVE 2x mode); slot order ORD
                    # matches attn's slots. y is accumulated in two slot-halves
                    # so DVE overlaps the V matmuls instead of trailing them.
                    v_th = [
                        ybp.tile([128, 512, HB], F16, name=f"v_t{ha}",
                                 tag=f"v_t{ha}")
                        for ha in range(2)
                    ]
                    yt8 = ybp.tile([128, NH, HD, HB], F16, name="yt8",
                                   tag="yt8", bufs=1)
                    ysc1 = ybp.tile([128, NH * HD, 2], F16, name="ysc1",
                                    tag="ysc1", bufs=1)
                    yhalf = [
                        ybp.tile([128, DIM], F16, name=f"yh{ha}", tag=f"yh{ha}",
                                 bufs=1)
                        for ha in range(2)
                    ]

                    def emit_y_half(ha):
                        sl = slice(HB * ha, HB * (ha + 1))
                        nc.vector.tensor_mul(
                            yt8[:].rearrange("p h j d -> p (h j d)"),
                            v_th[ha][:]
                            .rearrange("p (g jx) d -> p g (jx d)", g=NKV)
                            .unsqueeze(2)
                            .broadcast_to((128, NKV, 4, HD * HB)),
                            attn[:, :, sl]
                            .unsqueeze(2)
                            .broadcast_to((128, NH, HD, HB)),
                        )
                        f0 = yt8[:].rearrange("p h j d -> p (h j) d")
                        nc.vector.tensor_add(
                            ysc1[:], f0[:, :, 0:2], f0[:, :, 2:4]
                        )
                        nc.vector.tensor_add(
                            yhalf[ha][:].unsqueeze(2),
                            ysc1[:, :, 0:1],
                            ysc1[:, :, 1:2],
                        )

                    for j, dep in enumerate(ORD):
                        kvt = kvbp.tile([128, DC, 128], F16, name="kvtb", tag="kvtb")
                        nc.sync.dma_start(kvt[:], kvt_d[dep, t])
                        vp = vps.tile([128, 512], F32, name="v_ps", tag="vps")
                        for c in range(DC):
                            nc.tensor.matmul(
                                vp[:],
                                kvt[:, c, :],
                                wv_sb[:, c, :],
                                start=(c == 0),
                                stop=(c == DC - 1),
                            )
                        nc.scalar.copy(
                            v_th[j // HB][:, :, j % HB : j % HB + 1], vp[:]
                        )
                        if j == HB - 1:
                            emit_y_half(0)
                    emit_y_half(1)
                    y = ybp.tile([128, DIM], F16, name="y", tag="y")
                    y_of[t] = y
                    nc.vector.tensor_add(y[:], yhalf[0][:], yhalf[1][:])
                    if t == 0:
                        for og in range(4):
                            nc.sync.dma_start(
                                wp_sb[:, :, og * 512 : (og + 1) * 512],
                                wp_d[:, :, og * 512 : (og + 1) * 512],
                            )
                    if t >= 1:
                        emit_transpose(t - 1)
                        emit_oproj(t - 1)
                emit_transpose(NT - 1)
                emit_oproj(NT - 1)
    nc.finalize()
    return nc


# ---------------------------------------------------------------- pjrt runner
class _Runner:
    """Persistent jitted shard_map executor (no output donation, so the
    compiled callable can be re-invoked for timing)."""

    def __init__(self, nc, n_cores):
        import jax
        from jax.experimental.shard_map import shard_map
        from jax.sharding import Mesh, NamedSharding, PartitionSpec

        from concourse import bass2jax

        bass2jax.install_neuronx_cc_hook()
        self.jax = jax
        self.nc = nc
        self.n_cores = n_cores

        in_names, out_names, out_avals = [], [], []
        partition_name = (
            nc.partition_id_tensor.name if nc.partition_id_tensor else None
        )
        for alloc in nc.m.functions[0].allocations:
            if not isinstance(alloc, mybir.MemoryLocationSet):
                continue
            name = alloc.memorylocations[0].name
            if alloc.kind == "ExternalInput":
                if name != partition_name:
                    in_names.append(name)
            elif alloc.kind == "ExternalOutput":
                out_names.append(name)
                shape = tuple(alloc.tensor_shape)
                dtype = mybir.dt.np(alloc.dtype)
                out_avals.append(jax.core.ShapedArray(shape, dtype))
        self.param_names = list(in_names)
        self.out_names = list(out_names)
        self.out_avals = out_avals
        all_in_names = in_names + out_names
        if partition_name is not None:
            all_in_names.append(partition_name)

        def _body(*args):
            operands = list(args)
            if partition_name is not None:
                operands.append(bass2jax.partition_id_tensor())
            outs = bass2jax._bass_exec_p.bind(
                *operands,
                out_avals=tuple(out_avals),
                in_names=tuple(all_in_names),
                out_names=tuple(out_names),
                lowering_input_output_aliases=(),
                sim_require_finite=True,
                sim_require_nnan=True,
                nc=nc,
            )
            return tuple(outs)

        devices = jax.devices()[:n_cores]
        assert len(devices) == n_cores
        self.mesh = Mesh(np.asarray(devices), ("core",))
        spec = PartitionSpec("core")
        n_all = len(self.param_names) + len(out_names)
        self.sharding = NamedSharding(self.mesh, spec)
        self.fn = jax.jit(
            shard_map(
                _body,
                mesh=self.mesh,
                in_specs=(spec,) * n_all,
                out_specs=(spec,) * len(out_names),
                check_rep=False,
            ),
            keep_unused=True,
        )
        self.dev_args = None

    def put(self, in_maps):
        jax = self.jax
        concat = [
            np.concatenate([np.asarray(m[name]) for m in in_maps], axis=0)
            for name in self.param_names
        ]
        zeros = [
            np.zeros((self.n_cores * a.shape[0], *a.shape[1:]), a.dtype)
            for a in self.out_avals
        ]
        self.dev_args = [
            jax.device_put(a, self.sharding) for a in (concat + zeros)
        ]
        jax.block_until_ready(self.dev_args)

    def run(self):
        outs = self.fn(*self.dev_args)
        self.jax.block_until_ready(outs)
        return outs

    def time_exec(self, iters=20):
        import time as _t

        self.run()  # warm
        times = []
        for _ in range(iters):
            t0 = _t.perf_counter()
            self.run()
            times.append(_t.perf_counter() - t0)
        return times


_RUNNER = None


# ---------------------------------------------------------------- entry point
def _make_in_maps(inputs_kvt, consts, weights, npc):
    shared = dict(weights)
    shared.update(consts)
    NT = npc // 128
    in_maps = []
    for core in range(NCORES):
        m = dict(shared)
        m["kvt"] = np.ascontiguousarray(
            inputs_kvt[:, core * NT : (core + 1) * NT]
        )
        in_maps.append(m)
    return in_maps


def kernel(x, depth_history, Wq, Wk, Wv, Wproj, q_gain):
    global _RUNNER
    x = np.asarray(x, np.float32)
    depth_history = np.asarray(depth_history, np.float32)
    B, S, D = x.shape
    N = B * S
    npc = N // NCORES

    consts = _host_constants(np.asarray(q_gain, np.float32))
    weights = _prep_weights(Wq, Wk, Wv, Wproj)
    kvt = _prep_kvt(x, depth_history)  # [DEP, NTG, 128, DC, 128]

    key = npc
    if key not in _PROGRAM_CACHE:
        _PROGRAM_CACHE[key] = build_program(npc)
    nc = _PROGRAM_CACHE[key]

    in_maps = _make_in_maps(kvt, consts, weights, npc)

    if _RUNNER is None or _RUNNER.nc is not nc:
        _RUNNER = _Runner(nc, NCORES)
    _RUNNER.put(in_maps)
    try:
        outs = _RUNNER.run()
    except Exception:
        # wedged device / transient axon failure: rebuild runner, retry once
        _RUNNER = _Runner(nc, NCORES)
        _RUNNER.put(in_maps)
        outs = _RUNNER.run()
    oidx = _RUNNER.out_names.index("out")
    out = np.asarray(outs[oidx])
    return out.reshape(B, S, D).astype(np.float32)


# revision 4
# speedup vs baseline: 1.9274x; 1.9274x over previous
"""DepthSelfAttention Trainium2 kernel — position-major rewrite.

8-core data-parallel SPMD. Contract: kernel(**inputs) takes FULL unsharded
numpy inputs, returns the FULL [4, 2048, 2048] fp32 output.

Layout strategy (vs the combo-major baseline):
  * positions on the partition dim everywhere (tiles of 128 positions)
  * TensorE does ONLY the 4 projections (+16 small transposes/tile)
  * rmsnorm / RoPE / scores / softmax / attn*V run on DVE+Scalar with
    broadcast access patterns -> no auxiliary matmuls
  * DVE work is batched across all 8 depths per tile, f16 contiguous
    outputs keep the DVE in its 2x/4x fast modes
  * two passes per core: A = Q+K+scores+attn (wq,wk resident),
    B = V+y+yT+O per tile with lag-1 skew (wv+wp resident)
"""

import sys

sys.path.insert(0, "/opt/trn_rl_repo")

from contextlib import ExitStack, nullcontext

import numpy as np

import concourse.bass as bass
import concourse.tile as tile
from concourse import bacc, mybir

F16 = mybir.dt.float16
F32 = mybir.dt.float32

DIM = 2048
NH = 16
NKV = 4
HD = 128
DEP = 8  # 7 history + current
NCORES = 8
EPS = 1.1920929e-07
SCALE = 1.0 / float(np.sqrt(HD))
ROPE_BASE = 10000.0
MAX_DEPTH = 16
DC = DIM // 128  # 16 contraction chunks
# depth processing order: slot j holds depth ORD[j]; depth 7 (= x itself)
# first because its kvt tile doubles as the Q-projection input
ORD = [7, 0, 1, 2, 3, 4, 5, 6]

LAST_RESULTS = None
_PROGRAM_CACHE = {}


# ---------------------------------------------------------------- host tables
def _rope_tables():
    inv_freq = 1.0 / ROPE_BASE ** (
        np.arange(0, HD, 2, dtype=np.float32) / HD
    )  # [64]
    pos = np.arange(MAX_DEPTH, dtype=np.float32)
    rpos = np.arange(MAX_DEPTH - 1, -1, -1, dtype=np.float32)
    fw = np.outer(pos, inv_freq)
    rv = np.outer(rpos, inv_freq)
    return np.cos(fw), np.sin(fw), np.cos(rv), np.sin(rv)  # each [16, 64]


def _host_constants(q_gain):
    DCOS, DSIN, RCOS, RSIN = _rope_tables()
    d = DEP - 1  # query position index

    # pair-table layout: rot = [x1*c + x2*s | x1*nrs + x2*rc]  (nrs = -rev_sin)
    #   tabA = [c | nrs] multiplies x1 for both halves,
    #   tabB = [s | rc ] multiplies x2 for both halves.
    qtab = np.stack(
        [np.concatenate([DCOS[d], -RSIN[d]]), np.concatenate([DSIN[d], RCOS[d]])]
    )  # [2(A/B), 128]
    qtab = np.broadcast_to(qtab, (128, 2, HD))

    ktab = np.stack(
        [
            np.concatenate([DCOS[:DEP], -RSIN[:DEP]], axis=1),
            np.concatenate([DSIN[:DEP], RCOS[:DEP]], axis=1),
        ]
    )  # [2(A/B), 8, 128]
    ktab = ktab[:, ORD, :]  # slot order (slot j holds depth ORD[j])
    ktab = np.broadcast_to(ktab, (128, 2, DEP, HD))

    ident = np.eye(128, dtype=np.float32)

    cg = SCALE * np.asarray(q_gain, np.float64)  # [16]
    cg = np.broadcast_to(cg, (128, NH))

    f16 = np.float16
    return {
        "qtab": np.ascontiguousarray(qtab).astype(f16),
        "ktab": np.ascontiguousarray(ktab).astype(f16),
        "ident": ident.astype(f16),
        "cg": np.ascontiguousarray(cg).astype(np.float32),
        "epsb": np.full((128, 1), EPS, np.float32),
    }


def _prep_weights(Wq, Wk, Wv, Wproj):
    f16 = np.float16

    def t_part(w):  # [out, din] -> [128, din//128, out]
        wt = np.ascontiguousarray(np.asarray(w, np.float32).T)  # [din, out]
        no = wt.shape[1]
        return np.ascontiguousarray(
            wt.reshape(wt.shape[0] // 128, 128, no).transpose(1, 0, 2)
        ).astype(f16)

    return {
        "wq": t_part(Wq),  # [128, 16, 2048]
        "wk": t_part(Wk),  # [128, 16, 512]
        "wv": t_part(Wv),  # [128, 16, 512]
        "wp": t_part(Wproj),  # [128, 16, 2048]
    }


def _prep_kvt(x, depth_history):
    """-> kvt [DEP, NTG, 128p, DC, 128n] fp16 where NTG = N/128 global tiles.

    kvt[dep, T, p, c, n] = kv[T*128+n, dep, c*128+p]
    """
    B, S, D = x.shape
    N = B * S
    xf = np.asarray(x, np.float32).reshape(N, 1, D)
    dh = np.asarray(depth_history, np.float32).reshape(N, DEP - 1, D)
    kv = np.concatenate([dh, xf], axis=1).astype(np.float16)  # [N, 8, D]
    NTG = N // 128
    kvt = (
        kv.transpose(1, 0, 2)  # [DEP, N, D]
        .reshape(DEP, NTG, 128, DC, 128)  # [dep, T, n, c, p]
        .transpose(0, 1, 4, 3, 2)  # [dep, T, p, c, n]
    )
    return np.ascontiguousarray(kvt)


# ---------------------------------------------------------------- device code
def build_program(npc, reps=1):
    """One-core SPMD program for npc positions (NT = npc/128 tiles)."""
    NT = npc // 128
    assert npc % 128 == 0

    nc = bacc.Bacc()
    kvt_d = nc.declare_dram_parameter(
        "kvt", [DEP, NT, 128, DC, 128], F16, isOutput=False
    )
    wq_d = nc.declare_dram_parameter("wq", [128, DC, DIM], F16, isOutput=False)
    wk_d = nc.declare_dram_parameter("wk", [128, DC, 512], F16, isOutput=False)
    wv_d = nc.declare_dram_parameter("wv", [128, DC, 512], F16, isOutput=False)
    wp_d = nc.declare_dram_parameter("wp", [128, DC, DIM], F16, isOutput=False)
    qtab_d = nc.declare_dram_parameter("qtab", [128, 2, HD], F16, isOutput=False)
    ktab_d = nc.declare_dram_parameter(
        "ktab", [128, 2, DEP, HD], F16, isOutput=False
    )
    id_d = nc.declare_dram_parameter("ident", [128, 128], F16, isOutput=False)
    cg_d = nc.declare_dram_parameter("cg", [128, NH], F32, isOutput=False)
    epsb_d = nc.declare_dram_parameter("epsb", [128, 1], F32, isOutput=False)
    out_d = nc.declare_dram_parameter("out", [npc, DIM], F32, isOutput=True)

    AF = mybir.ActivationFunctionType
    AX = mybir.AxisListType
    OP = mybir.AluOpType

    with tile.TileContext(nc) as tc, ExitStack() as top:
        const = top.enter_context(tc.tile_pool(name="const", bufs=1))

        def load_const(dram, shape, name):
            t = const.tile(shape, dram.dtype, name=name)
            nc.sync.dma_start(t[:], dram[:])
            return t

        qtab_sb = load_const(qtab_d, [128, 2, HD], "qtab_sb")
        ktab_sb = load_const(ktab_d, [128, 2, DEP, HD], "ktab_sb")
        id_sb = load_const(id_d, [128, 128], "id_sb")
        cg_sb = load_const(cg_d, [128, NH], "cg_sb")
        epsb_sb = load_const(epsb_d, [128, 1], "epsb_sb")

        rep_ctx = tc.For_i(0, reps, 1) if reps > 4 else nullcontext(None)
        with rep_ctx:
         for _rep in range(reps if reps <= 4 else 1):
          with (
              tc.tile_pool(name="keep", bufs=1) as keep,
              tc.tile_pool(name="wvp", bufs=1) as wvp,
              tc.tile_pool(name="kvbp", bufs=3) as kvbp,
          ):
            attn_keep = {}
            # wv lives in a rep-level pool (disjoint from the phase-A pools)
            # so its DMA (emitted at tile1) overlaps phase A instead of
            # gating phase B's start
            wv_sb = wvp.tile([128, DC, 512], F16, name="wv_sb")

            # ================= Phase A: Q + K + scores + attn =================
            with (
                tc.tile_pool(name="wqp", bufs=1) as wqp,
                tc.tile_pool(name="wkp", bufs=1) as wkp,
                tc.tile_pool(name="kv7p", bufs=2) as kv7p,
                tc.tile_pool(name="kvp", bufs=3) as kvp,
                tc.tile_pool(name="qps", bufs=4, space="PSUM") as qps,
                tc.tile_pool(name="kps", bufs=4, space="PSUM") as kps,
                tc.tile_pool(name="qsp", bufs=2) as qsp,
                tc.tile_pool(name="wrk", bufs=1) as wrk,
            ):
                # DMA issue order tuned for the serial queue: tile0's kv7
                # first, then wq in chunks interleaved with wk, so tile0's
                # Q-proj starts after ~2 chunks instead of after all weights.
                wq_sb = wqp.tile([128, DC, DIM], F16, name="wq_sb")
                wk_sb = wkp.tile([128, DC, 512], F16, name="wk_sb")
                kv7_first = kv7p.tile([128, DC, 128], F16, name="kv7", tag="kv7")
                nc.sync.dma_start(kv7_first[:], kvt_d[DEP - 1, 0])
                for oc in range(4):
                    nc.sync.dma_start(
                        wq_sb[:, :, oc * 512 : (oc + 1) * 512],
                        wq_d[:, :, oc * 512 : (oc + 1) * 512],
                    )
                    if oc == 1:
                        nc.sync.dma_start(wk_sb[:], wk_d[:])

                for t in range(NT):
                    if t == 0:
                        kv7 = kv7_first
                    else:
                        kv7 = kv7p.tile([128, DC, 128], F16, name="kv7", tag="kv7")
                        nc.sync.dma_start(kv7[:], kvt_d[DEP - 1, t])

                    q_sb = qsp.tile([128, DIM], F16, name="q_sb", tag="q")
                    qsq = wrk.tile([128, DEP // 2, NH, HD], F16, name="qsq",
                                   tag="big")
                    t_a = wrk.tile([128, DEP * 512], F16, name="t_a", tag="t_a")
                    t_b = wrk.tile([128, DEP * 512], F16, name="t_b", tag="t_b")
                    qrot = qsp.tile([128, NH, HD], F16, name="qrot", tag="qrot",
                                    bufs=1)
                    k_all = qsp.tile([128, DEP, 512], F16, name="k_all", tag="kall")
                    ksq = wrk.tile([128, DEP, 512], F16, name="ksq", tag="t_c")
                    ssq_k = wrk.tile([128, DEP, NKV], F16, name="ssq_k", tag="sk")
                    scores = wrk.tile([128, DEP, NH], F16, name="scores", tag="sc")
                    dq = wrk.tile([128, NH], F32, name="dq", tag="dq")
                    HB = DEP // 2  # slots per half-batch

                    def emit_q():
                        # ---- Q projection: q_sb [128 pos, 2048] f16 ----
                        for oc in range(4):
                            qp = qps.tile([128, 512], F32, name="q_ps", tag="qps")
                            for c in range(DC):
                                nc.tensor.matmul(
                                    qp[:],
                                    kv7[:, c, :],
                                    wq_sb[:, c, oc * 512 : (oc + 1) * 512],
                                    start=(c == 0),
                                    stop=(c == DC - 1),
                                )
                            nc.scalar.copy(
                                q_sb[:, oc * 512 : (oc + 1) * 512], qp[:]
                            )

                        # ---- q rmsnorm stats + rope (DVE, f16 fast modes) ----
                        qsq_v = qsq[:].rearrange("p d h x -> p (d h x)")[:, :DIM]
                        nc.scalar.square(qsq_v, q_sb[:])
                        ssq_q = wrk.tile([128, NH], F16, name="ssq_q", tag="ssq_q")
                        with nc.allow_low_precision("f16 sumsq, values O(100)"):
                            nc.vector.tensor_reduce(
                                ssq_q[:],
                                qsq_v.rearrange("p (h x) -> p h x", h=NH),
                                axis=AX.X,
                                op=OP.add,
                            )
                        nc.scalar.activation(
                            dq[:], ssq_q[:], AF.Sqrt, bias=epsb_sb[:],
                            scale=1.0 / HD,
                        )

                        q_v = q_sb[:].rearrange(
                            "p (h two j) -> p h two j", two=2, j=64
                        )
                        t_qa = t_a[:, :DIM].rearrange("p (h x) -> p h x", h=NH)
                        t_qb = t_b[:, :DIM].rearrange("p (h x) -> p h x", h=NH)
                        nc.vector.tensor_mul(
                            t_qa,
                            q_v[:, :, 0:1, :].broadcast_to((128, NH, 2, 64)),
                            qtab_sb[:, 0:1, :].broadcast_to((128, NH, HD)),
                        )
                        nc.vector.tensor_mul(
                            t_qb,
                            q_v[:, :, 1:2, :].broadcast_to((128, NH, 2, 64)),
                            qtab_sb[:, 1:2, :].broadcast_to((128, NH, HD)),
                        )
                        nc.vector.tensor_add(qrot[:], t_qa, t_qb)

                    def emit_kdve_half(ha):
                        sl = slice(HB * ha, HB * (ha + 1))
                        nc.scalar.square(ksq[:, sl, :], k_all[:, sl, :])
                        with nc.allow_low_precision("f16 sumsq, values O(100)"):
                            nc.vector.tensor_reduce(
                                ssq_k[:, sl, :],
                                ksq[:, sl, :].rearrange(
                                    "p d (g x) -> p d g x", g=NKV
                                ),
                                axis=AX.X,
                                op=OP.add,
                            )
                        # walrus DVE ISA allows at most 3 free AP dims, so
                        # (d,g) is pre-merged into one dim everywhere below
                        k_v = k_all[:, sl, :].rearrange(
                            "p d (g two j) -> p (d g) two j", two=2, j=64
                        )
                        t_ka = t_a[:, : HB * 512]
                        t_kb = t_b[:, : HB * 512]
                        nc.vector.tensor_mul(
                            t_ka,
                            k_v[:, :, 0:1, :].broadcast_to(
                                (128, HB * NKV, 2, 64)
                            ),
                            ktab_sb[:, 0, sl, :].unsqueeze(2).broadcast_to(
                                (128, HB, NKV, HD)
                            ),
                        )
                        nc.vector.tensor_mul(
                            t_kb,
                            k_v[:, :, 1:2, :].broadcast_to(
                                (128, HB * NKV, 2, 64)
                            ),
                            ktab_sb[:, 1, sl, :].unsqueeze(2).broadcast_to(
                                (128, HB, NKV, HD)
                            ),
                        )
                        # krot overwrites ksq's slots (already consumed)
                        krot_flat = ksq[:, sl, :].rearrange("p d x -> p (d x)")
                        nc.vector.tensor_add(krot_flat, t_ka, t_kb)

                        prod = qsq[:, :HB, :, :]  # big tmp tile region
                        nc.vector.tensor_mul(
                            prod.rearrange("p d h x -> p (d h x)"),
                            qrot[:]
                            .rearrange("p h x -> p (h x)")
                            .unsqueeze(1)
                            .broadcast_to((128, HB, NH * HD)),
                            ksq[:, sl, :]
                            .rearrange("p d (g x) -> p (d g) x", g=NKV)
                            .unsqueeze(2)
                            .broadcast_to((128, HB * NKV, 4, HD)),
                        )
                        # tree-fold the 128-wide dot products (2x mode; a
                        # single tensor_reduce runs at 1x and is ~2x slower)
                        fsrc = prod.rearrange("p d h x -> p (d h) x")
                        width = HD
                        ping, pong = t_a, t_b
                        while width > 2:
                            half = width // 2
                            fdst = ping[:, : HB * NH * half].rearrange(
                                "p (c x) -> p c x", c=HB * NH
                            )
                            nc.vector.tensor_add(
                                fdst, fsrc[:, :, :half], fsrc[:, :, half:width]
                            )
                            fsrc = fdst
                            ping, pong = pong, ping
                            width = half
                        nc.vector.tensor_add(
                            scores[:, sl, :]
                            .rearrange("p d h -> p (d h)")
                            .unsqueeze(2),
                            fsrc[:, :, 0:1],
                            fsrc[:, :, 1:2],
                        )

                    def emit_kmm(mid_dve):
                        # ---- K projections (8 depths, slot order ORD) ----
                        for j, dep in enumerate(ORD):
                            if j == 0:
                                kvt = kv7
                            else:
                                kvt = kvp.tile(
                                    [128, DC, 128], F16, name="kvt", tag="kvt"
                                )
                                nc.sync.dma_start(kvt[:], kvt_d[dep, t])
                            kp = kps.tile([128, 512], F32, name="k_ps", tag="kps")
                            for c in range(DC):
                                nc.tensor.matmul(
                                    kp[:],
                                    kvt[:, c, :],
                                    wk_sb[:, c, :],
                                    start=(c == 0),
                                    stop=(c == DC - 1),
                                )
                            nc.scalar.copy(k_all[:, j, :], kp[:])
                            if mid_dve and j == HB - 1:
                                emit_kdve_half(0)

                    if t == 0:
                        # tile0: K first so PE starts on wk+kv7 while the wq
                        # chunks are still streaming in; DVE K-halves run
                        # after Q (they need qrot)
                        emit_kmm(mid_dve=False)
                        emit_q()
                        emit_kdve_half(0)
                        emit_kdve_half(1)
                    else:
                        emit_q()
                        emit_kmm(mid_dve=True)
                        emit_kdve_half(1)
                    if t == min(1, NT - 1):
                        # wv prefetch; queue-positioned here so it does not
                        # delay the startup-critical wq/kv7 transfers
                        nc.sync.dma_start(wv_sb[:], wv_d[:])

                    # ---- softmax over depth (deferred rms normalization) ----
                    dk = wrk.tile([128, DEP, NKV], F32, name="dk", tag="dk")
                    nc.scalar.activation(
                        dk[:], ssq_k[:], AF.Sqrt, bias=epsb_sb[:], scale=1.0 / HD
                    )
                    dprod = wrk.tile([128, NH, DEP], F32, name="dprod", tag="dp")
                    nc.vector.tensor_mul(
                        dprod[:].rearrange("p (g r) x -> p g r x", g=NKV),
                        dq[:]
                        .rearrange("p (g r) -> p g r", g=NKV)
                        .unsqueeze(3)
                        .broadcast_to((128, NKV, 4, DEP)),
                        dk[:]
                        .rearrange("p d g -> p g d")
                        .unsqueeze(2)
                        .broadcast_to((128, NKV, 4, DEP)),
                    )
                    rden = wrk.tile([128, NH, DEP], F32, name="rden", tag="rd")
                    nc.vector.reciprocal(rden[:], dprod[:])
                    factor = wrk.tile([128, NH, DEP], F32, name="factor", tag="fa")
                    nc.vector.tensor_mul(
                        factor[:],
                        rden[:],
                        cg_sb[:].unsqueeze(2).broadcast_to((128, NH, DEP)),
                    )
                    scaled = wrk.tile([128, NH, DEP], F32, name="scaled", tag="sd")
                    nc.vector.tensor_mul(
                        scaled[:],
                        scores[:].rearrange("p d h -> p h d"),
                        factor[:],
                    )
                    epx = wrk.tile([128, NH, DEP], F32, name="epx", tag="ep")
                    nc.scalar.activation(epx[:], scaled[:], AF.Exp)
                    dsum = wrk.tile([128, NH], F32, name="dsum", tag="ds")
                    nc.vector.tensor_reduce(
                        dsum[:], epx[:], axis=AX.X, op=OP.add
                    )
                    rsum = wrk.tile([128, NH], F32, name="rsum", tag="rs")
                    nc.vector.reciprocal(rsum[:], dsum[:])
                    attn = keep.tile(
                        [128, NH, DEP], F16, name=f"attn{t}", tag=f"attn{t}"
                    )
                    attn_keep[t] = attn
                    nc.vector.tensor_mul(
                        attn[:],
                        epx[:],
                        rsum[:].unsqueeze(2).broadcast_to((128, NH, DEP)),
                    )

            # ============ Phase B: V + y + yT + O (lag-1 skew) ============
            # pool-open order matters: ybp first so it lands in wq's freed
            # region (whose users finish early) rather than overlapping the
            # wrk region still being read by phase A's DVE tail; vps first
            # among PSUM pools so it lands on banks phase A never used.
            with (
                tc.tile_pool(name="vps", bufs=2, space="PSUM") as vps,
                tc.tile_pool(name="tps", bufs=2, space="PSUM") as tps,
                tc.tile_pool(name="ybp", bufs=2) as ybp,
                tc.tile_pool(name="wpp", bufs=1) as wpp,
                tc.tile_pool(name="ops", bufs=2, space="PSUM") as opsp,
                tc.tile_pool(name="osb", bufs=3) as osbp,
            ):
                # wp prefetch is emitted in chunks after tile0's kvt loads so
                # it doesn't block the V-path stream on the serial DMA queue;
                # first use (O-proj of tile0) is one tile later.
                wp_sb = wpp.tile([128, DC, DIM], F16, name="wp_sb")

                y_of = {}
                yt_of = {}

                def emit_transpose(t):
                    y = y_of.pop(t)
                    tp = tps.tile([128, NH, 128], F16, name="t_ps", tag="tps")
                    for h in range(NH):
                        nc.tensor.transpose(
                            tp[:, h, :], y[:, h * 128 : (h + 1) * 128], id_sb[:]
                        )
                    yt = ybp.tile([128, DC, 128], F16, name="yt", tag="yt")
                    yt_of[t] = yt
                    nc.scalar.copy(yt[:], tp[:])

                def emit_oproj(t):
                    yt = yt_of.pop(t)
                    row = t * 128
                    for og in range(4):
                        op_t = opsp.tile([128, 512], F32, name="o_ps", tag="ops")
                        for c in range(DC):
                            nc.tensor.matmul(
                                op_t[:],
                                yt[:, c, :],
                                wp_sb[:, c, og * 512 : (og + 1) * 512],
                                start=(c == 0),
                                stop=(c == DC - 1),
                            )
                        ost = osbp.tile([128, 512], F32, name="ost", tag="ost")
                        nc.scalar.copy(ost[:], op_t[:])
                        nc.sync.dma_start(
                            out_d[row : row + 128, og * 512 : (og + 1) * 512],
                            ost[:],
                        )

                HB = DEP // 2
                for t in range(NT):
                    attn = attn_keep[t]
                    # v_t: depth-innermost V staging so the big attn*v multiply
                    # keeps a stride-1 last dim (DVE 2x mode); slot order ORD
                    # matches attn's slots. y is accumulated in two slot-halves
                    # so DVE overlaps the V matmuls instead of trailing them.
                    v_th = [
                        ybp.tile([128, 512, HB], F16, name=f"v_t{ha}",
                                 tag=f"v_t{ha}")
                        for ha in range(2)
                    ]
                    yt8 = ybp.tile([128, NH, HD, HB], F16, name="yt8",
                                   tag="yt8", bufs=1)
                    ysc1 = ybp.tile([128, NH * HD, 2], F16, name="ysc1",
                                    tag="ysc1", bufs=1)
                    yhalf = [
                        ybp.tile([128, DIM], F16, name=f"yh{ha}", tag=f"yh{ha}",
                                 bufs=1)
                        for ha in range(2)
                    ]

                    def emit_y_half(ha):
                        sl = slice(HB * ha, HB * (ha + 1))
                        nc.vector.tensor_mul(
                            yt8[:].rearrange("p h j d -> p (h j d)"),
                            v_th[ha][:]
                            .rearrange("p (g jx) d -> p g (jx d)", g=NKV)
                            .unsqueeze(2)
                            .broadcast_to((128, NKV, 4, HD * HB)),
                            attn[:, :, sl]
                            .unsqueeze(2)
                            .broadcast_to((128, NH, HD, HB)),
                        )
                        f0 = yt8[:].rearrange("p h j d -> p (h j) d")
                        nc.vector.tensor_add(
                            ysc1[:], f0[:, :, 0:2], f0[:, :, 2:4]
                        )
                        nc.vector.tensor_add(
                            yhalf[ha][:].unsqueeze(2),
                            ysc1[:, :, 0:1],
                            ysc1[:, :, 1:2],
                        )

                    for j, dep in enumerate(ORD):
                        kvt = kvbp.tile([128, DC, 128], F16, name="kvtb", tag="kvtb")
                        nc.sync.dma_start(kvt[:], kvt_d[dep, t])
                        vp = vps.tile([128, 512], F32, name="v_ps", tag="vps")
                        for c in range(DC):
                            nc.tensor.matmul(
                                vp[:],
                                kvt[:, c, :],
                                wv_sb[:, c, :],
                                start=(c == 0),
                                stop=(c == DC - 1),
                            )
                        nc.scalar.copy(
                            v_th[j // HB][:, :, j % HB : j % HB + 1], vp[:]
                        )
                        if j == HB - 1:
                            emit_y_half(0)
                    emit_y_half(1)
                    y = ybp.tile([128, DIM], F16, name="y", tag="y")
                    y_of[t] = y
                    nc.vector.tensor_add(y[:], yhalf[0][:], yhalf[1][:])
                    if t == 0:
                        for og in range(4):
                            nc.sync.dma_start(
                                wp_sb[:, :, og * 512 : (og + 1) * 512],
                                wp_d[:, :, og * 512 : (og + 1) * 512],
                            )
                    if t >= 1:
                        emit_transpose(t - 1)
                        emit_oproj(t - 1)
                emit_transpose(NT - 1)
                emit_oproj(NT - 1)
    nc.finalize()
    return nc


# ---------------------------------------------------------------- pjrt runner
class _Runner:
    """Persistent jitted shard_map executor (no output donation, so the
    compiled callable can be re-invoked for timing)."""

    def __init__(self, nc, n_cores):
        import jax
        from jax.experimental.shard_map import shard_map
        from jax.sharding import Mesh, NamedSharding, PartitionSpec

        from concourse import bass2jax

        bass2jax.install_neuronx_cc_hook()
        self.jax = jax
        self.nc = nc
        self.n_cores = n_cores

        in_names, out_names, out_avals = [], [], []
        partition_name = (
            nc.partition_id_tensor.name if nc.partition_id_tensor else None
        )
        for alloc in nc.m.functions[0].allocations:
            if not isinstance(alloc, mybir.MemoryLocationSet):
                continue
            name = alloc.memorylocations[0].name
            if alloc.kind == "ExternalInput":
                if name != partition_name:
                    in_names.append(name)
            elif alloc.kind == "ExternalOutput":
                out_names.append(name)
                shape = tuple(alloc.tensor_shape)
                dtype = mybir.dt.np(alloc.dtype)
                out_avals.append(jax.core.ShapedArray(shape, dtype))
        self.param_names = list(in_names)
        self.out_names = list(out_names)
        self.out_avals = out_avals
        all_in_names = in_names + out_names
        if partition_name is not None:
            all_in_names.append(partition_name)

        def _body(*args):
            operands = list(args)
            if partition_name is not None:
                operands.append(bass2jax.partition_id_tensor())
            outs = bass2jax._bass_exec_p.bind(
                *operands,
                out_avals=tuple(out_avals),
                in_names=tuple(all_in_names),
                out_names=tuple(out_names),
                lowering_input_output_aliases=(),
                sim_require_finite=True,
                sim_require_nnan=True,
                nc=nc,
            )
            return tuple(outs)

        devices = jax.devices()[:n_cores]
        assert len(devices) == n_cores
        self.mesh = Mesh(np.asarray(devices), ("core",))
        spec = PartitionSpec("core")
        n_all = len(self.param_names) + len(out_names)
        self.sharding = NamedSharding(self.mesh, spec)
        self.fn = jax.jit(
            shard_map(
                _body,
                mesh=self.mesh,
                in_specs=(spec,) * n_all,
                out_specs=(spec,) * len(out_names),
                check_rep=False,
            ),
            keep_unused=True,
        )
        self.dev_args = None

    def put(self, in_maps):
        jax = self.jax
        concat = [
            np.concatenate([np.asarray(m[name]) for m in in_maps], axis=0)
            for name in self.param_names
        ]
        zeros = [
            np.zeros((self.n_cores * a.shape[0], *a.shape[1:]), a.dtype)
            for a in self.out_avals
        ]
        self.dev_args = [
            jax.device_put(a, self.sharding) for a in (concat + zeros)
        ]
        jax.block_until_ready(self.dev_args)

    def run(self):
        outs = self.fn(*self.dev_args)
        self.jax.block_until_ready(outs)
        return outs

    def time_exec(self, iters=20):
        import time as _t

        self.run()  # warm
        times = []
        for _ in range(iters):
            t0 = _t.perf_counter()
            self.run()
            times.append(_t.perf_counter() - t0)
        return times


_RUNNER = None


# ---------------------------------------------------------------- entry point
def _make_in_maps(inputs_kvt, consts, weights, npc):
    shared = dict(weights)
    shared.update(consts)
    NT = npc // 128
    in_maps = []
    for core in range(NCORES):
        m = dict(shared)
        m["kvt"] = np.ascontiguousarray(
            inputs_kvt[:, core * NT : (core + 1) * NT]
        )
        in_maps.append(m)
    return in_maps


def kernel(x, depth_history, Wq, Wk, Wv, Wproj, q_gain):
    global _RUNNER
    x = np.asarray(x, np.float32)
    depth_history = np.asarray(depth_history, np.float32)
    B, S, D = x.shape
    N = B * S
    npc = N // NCORES

    consts = _host_constants(np.asarray(q_gain, np.float32))
    weights = _prep_weights(Wq, Wk, Wv, Wproj)
    kvt = _prep_kvt(x, depth_history)  # [DEP, NTG, 128, DC, 128]

    key = npc
    if key not in _PROGRAM_CACHE:
        _PROGRAM_CACHE[key] = build_program(npc)
    nc = _PROGRAM_CACHE[key]

    in_maps = _make_in_maps(kvt, consts, weights, npc)

    if _RUNNER is None or _RUNNER.nc is not nc:
        _RUNNER = _Runner(nc, NCORES)
    _RUNNER.put(in_maps)
    try:
        outs = _RUNNER.run()
    except Exception:
        # wedged device / transient axon failure: rebuild runner, retry once
        _RUNNER = _Runner(nc, NCORES)
        _RUNNER.put(in_maps)
        outs = _RUNNER.run()
    oidx = _RUNNER.out_names.index("out")
    out = np.asarray(outs[oidx])
    return out.reshape(B, S, D).astype(np.float32)
